# revision 16
# baseline (speedup 1.0000x reference)
"""Bounding-box kernel for Trainium2 (Bass/Tile), 8-core SPMD.

Problem: mask [128, 1, 512, 512] f32 -> bbox [128, 4] int32
  (y_min, x_min, y_max, x_max) of the region where mask >= 0.5,
  with (0, 0, H, W) when a row/col has no hit.

Strategy (per core, 16 images, single qSync HWDGE queue ~424 GB/s):
  - Images are loaded as 8 PAIR tiles [128, 8, 512]: tile t stacks
    image t (partitions 0-63) and image t+8 (partitions 64-127); each
    partition holds 8 contiguous DRAM rows -> 16KB DMA descriptors
    (halves descriptor count vs 8KB, trimming the queue-manager
    engine's overhead which straggles the stream tail).
  - Threshold: ACT computes h = Relu(x*2^25 - (2^24-1)) in bf16, which
    is exactly 0 iff x < 0.5 and >= 1 otherwise (exact for every f32).
  - Column extents: one-hot [128, 16] lhsT matmuls accumulate per-image
    column hit-mass for all 16 images into PSUM [16, 512]; gpsimd
    pre-adds subrow pairs to halve the matmul count.
  - Row extents: DVE reduce_max over W of h (bf16) -> rowmax [128, 64]
    (col = t*8 + j), compare >= 0.5, mul by per-(partition,subrow)
    index consts, reduce over j -> [128, 8], one PE transpose of the
    stacked [128, 16] lo|hi -> [16, 128] PSUM, then 4 narrow reduces
    over the image halves.
  - Last pair tile is loaded as four [128, 2, 512] quarter chunks so
    the final arrival->answer chain is short.
"""

import numpy as np
import ml_dtypes
from contextlib import ExitStack

import concourse.bass as bass
import concourse.bacc as bacc
import concourse.tile as tile
import concourse.mybir as mybir
from concourse.bass_utils import run_bass_kernel_spmd

N_CORES = 8
N, H, W = 128, 512, 512
NPC = N // N_CORES          # images per core = 16
P = 128                     # SBUF partitions
NPAIR = NPC // 2            # 8 pair tiles; tile t = images (t, t+8)
JROWS = 8                   # DRAM rows per partition in a pair tile
F32 = mybir.dt.float32
BF16 = mybir.dt.bfloat16
I32 = mybir.dt.int32

# Relu(x * 2^25 - (2^24 - 1)) == 0 iff x < 0.5, >= 1 iff x >= 0.5, exact
# for EVERY f32 x: x*2^25 is exact (power-of-2 scale); for x < 0.5,
# x*2^25 <= 2^24 - 1 so the true sum is <= 0 (rounding is monotone, 0 is
# representable); for x >= 0.5 the true sum is >= 1 and rounds to >= 1.
ACT_SCALE = float(2**25)
ACT_BIAS = float(1 - 2**24)

TRACE = False               # test.py sets True to capture a HW profile
LAST_RESULTS = None         # BassKernelResults of the last run

_compiled = None


def _build_nc():
    nc = bacc.Bacc(
        "TRN2", target_bir_lowering=False, debug=False, num_devices=N_CORES
    )
    mask_d = nc.dram_tensor("mask", [NPC * H, W], F32, kind="ExternalInput").ap()
    oneh_d = nc.dram_tensor("onehot", [P, NPAIR * NPC], BF16, kind="ExternalInput").ap()
    ident_d = nc.dram_tensor("ident", [P, P], F32, kind="ExternalInput").ap()
    xlo_d = nc.dram_tensor("xlo_const", [NPC, W], F32, kind="ExternalInput").ap()
    xhi_d = nc.dram_tensor("xhi_const", [NPC, W], F32, kind="ExternalInput").ap()
    # y index consts pre-masked by partition half: "L" is zero on
    # partitions 64-127 (image t), "H" is zero on partitions 0-63
    # (image t+8) — zeros are neutral for the min/max with 0-sentinels
    ylo_ds = [
        nc.dram_tensor(f"ylo_const{s}", [P, JROWS * NPAIR], F32, kind="ExternalInput").ap()
        for s in range(2)
    ]
    yhi_ds = [
        nc.dram_tensor(f"yhi_const{s}", [P, JROWS * NPAIR], F32, kind="ExternalInput").ap()
        for s in range(2)
    ]
    bbox_d = nc.dram_tensor("bbox", [NPC, 4], I32, kind="ExternalOutput").ap()

    # [16 imgs, 512 rows, 512 cols] view; pair tile t reads images t, t+8
    mask_v = mask_d.rearrange("(i h) w -> i h w", i=NPC)

    with tile.TileContext(nc) as tc, ExitStack() as ctx:
        consts = ctx.enter_context(tc.tile_pool(name="consts", bufs=1))
        xpool = ctx.enter_context(tc.tile_pool(name="x", bufs=4))
        hpool = ctx.enter_context(tc.tile_pool(name="h", bufs=4))
        hspool = ctx.enter_context(tc.tile_pool(name="hs", bufs=3))
        lastpool = ctx.enter_context(tc.tile_pool(name="last", bufs=4))
        small = ctx.enter_context(tc.tile_pool(name="small", bufs=1))
        scratch = ctx.enter_context(tc.tile_pool(name="scratch", bufs=2))
        psum = ctx.enter_context(tc.tile_pool(name="psum", bufs=1, space="PSUM"))

        oneh = consts.tile([P, NPAIR * NPC], BF16)
        nc.scalar.dma_start(out=oneh[:], in_=oneh_d)
        ident = consts.tile([P, P], F32)
        nc.scalar.dma_start(out=ident[:], in_=ident_d)
        xlo_c = consts.tile([NPC, W], F32)
        nc.scalar.dma_start(out=xlo_c[:], in_=xlo_d)
        xhi_c = consts.tile([NPC, W], F32)
        nc.scalar.dma_start(out=xhi_c[:], in_=xhi_d)
        ylo_cs = []
        yhi_cs = []
        for s in range(2):
            c = consts.tile([P, JROWS * NPAIR], F32)
            nc.scalar.dma_start(out=c[:], in_=ylo_ds[s])
            ylo_cs.append(c)
            c = consts.tile([P, JROWS * NPAIR], F32)
            nc.scalar.dma_start(out=c[:], in_=yhi_ds[s])
            yhi_cs.append(c)
        act_bias = consts.tile([P, 1], F32)
        nc.vector.memset(act_bias[:], ACT_BIAS)

        # rowmax[p, t*8 + j] = max over w of pair-tile t's subrow j
        # (image t if p < 64 else t+8; image row r = 8*(p%64) + j)
        rowmax = small.tile([P, NPAIR * JROWS], BF16)
        rowmax_v = rowmax.rearrange("p (t j) -> p t j", t=NPAIR)
        cnt_ps = psum.tile([NPC, W], F32)    # per-image column hit-mass
        tpsL = psum.tile([NPC, P], F32)      # transposed y-lo stage
        tpsH = psum.tile([NPC, P], F32)      # transposed y-hi stage

        for t in range(NPAIR - 1):
            x = xpool.tile([P, JROWS, W], F32, tag="x")
            # one DMA per image half (the SBUF partition dim cannot be
            # split inside a single DMA AP): 64 x 16KB descriptors each
            for s in range(2):
                nc.sync.dma_start(
                    out=x[s * 64:(s + 1) * 64],
                    in_=mask_v[t + s * NPAIR].rearrange("(p j) w -> p j w", j=JROWS),
                )
            h = hpool.tile([P, JROWS, W], BF16, tag="h")
            # two half-tile activations: finer pipelining on ACT
            for u in range(2):
                nc.scalar.activation(
                    h[:, 4 * u:4 * u + 4, :], x[:, 4 * u:4 * u + 4, :],
                    mybir.ActivationFunctionType.Relu,
                    bias=act_bias[:], scale=ACT_SCALE,
                )
            nc.vector.tensor_reduce(
                out=rowmax_v[:, t, :], in_=h[:],
                axis=mybir.AxisListType.X, op=mybir.AluOpType.max,
            )
            # pre-sum subrow pairs on gpsimd: halves the PE matmul count
            # (hit-mass stays 0 iff no hit)
            h_v = h.rearrange("p (m q) w -> p m q w", q=2)
            hs = hspool.tile([P, JROWS // 2, W], BF16)
            nc.gpsimd.tensor_add(hs[:], h_v[:, :, 0, :], h_v[:, :, 1, :])
            lhsT = oneh[:, t * NPC:(t + 1) * NPC]
            for m in range(JROWS // 2):
                nc.tensor.matmul(
                    cnt_ps[:, :], lhsT, hs[:, m, :],
                    start=(t == 0 and m == 0), stop=False,
                )

        # last pair tile: four quarter loads (8KB descriptors) so the
        # final arrival -> answer chain is short
        t = NPAIR - 1
        lhsT = oneh[:, t * NPC:(t + 1) * NPC]
        for q in range(4):
            x = lastpool.tile([P, 2, W], F32, tag="xq")
            for s in range(2):
                nc.sync.dma_start(
                    out=x[s * 64:(s + 1) * 64],
                    in_=mask_v[t + s * NPAIR]
                    .rearrange("(p j) w -> p j w", j=JROWS)[:, 2 * q:2 * q + 2, :],
                )
            h = lastpool.tile([P, 2, W], BF16, tag="hq")
            nc.scalar.activation(
                h[:], x[:], mybir.ActivationFunctionType.Relu,
                bias=act_bias[:], scale=ACT_SCALE,
            )
            nc.vector.tensor_reduce(
                out=rowmax_v[:, t, 2 * q:2 * q + 2], in_=h[:],
                axis=mybir.AxisListType.X, op=mybir.AluOpType.max,
            )
            for b in range(2):
                nc.tensor.matmul(
                    cnt_ps[:, :], lhsT, h[:, b, :],
                    start=False, stop=(q == 3 and b == 1),
                )

        # ---- Y extents in [128, *] space ----
        rowhit = small.tile([P, NPAIR * JROWS], F32)
        nc.vector.tensor_scalar(
            rowhit[:], rowmax[:], 0.5, None, mybir.AluOpType.is_ge
        )
        # loI[:, s*8 + t] = min over j of rowhit * ylo_masked_s, which is
        # nonzero only on the partition half holding image s*8 + t; the
        # 0-sentinel of the other half is neutral for the final min/max.
        loI = small.tile([P, NPC], F32)
        hiI = small.tile([P, NPC], F32)
        for s in range(2):
            prod = scratch.tile([P, NPAIR * JROWS], F32, tag="yprod")
            nc.vector.tensor_mul(prod[:], rowhit[:], ylo_cs[s][:])
            prod_v = prod.rearrange("p (t j) -> p t j", t=NPAIR)
            nc.vector.tensor_reduce(
                out=loI[:, s * NPAIR:(s + 1) * NPAIR], in_=prod_v[:],
                axis=mybir.AxisListType.X, op=mybir.AluOpType.min,
            )
            prod2 = scratch.tile([P, NPAIR * JROWS], F32, tag="yprod")
            nc.vector.tensor_mul(prod2[:], rowhit[:], yhi_cs[s][:])
            prod2_v = prod2.rearrange("p (t j) -> p t j", t=NPAIR)
            nc.vector.tensor_reduce(
                out=hiI[:, s * NPAIR:(s + 1) * NPAIR], in_=prod2_v[:],
                axis=mybir.AxisListType.X, op=mybir.AluOpType.max,
            )
        # transpose [128, 16] -> [16, 128], partition = image
        nc.tensor.matmul(
            tpsL[:, :], loI[:], ident[:],
            is_transpose=True, start=True, stop=True,
        )
        nc.tensor.matmul(
            tpsH[:, :], hiI[:], ident[:],
            is_transpose=True, start=True, stop=True,
        )

        # raw extents tile: col 0 = ylo, 1 = xlo, 2 = yhi, 3 = xhi
        # (lo values are lo-512 for hit, 0 for none; hi are hi+1 or 0)
        raw = small.tile([NPC, 4], F32)
        nc.vector.tensor_reduce(
            out=raw[:, 0:1], in_=tpsL[:, :],
            axis=mybir.AxisListType.X, op=mybir.AluOpType.min,
        )
        nc.vector.tensor_reduce(
            out=raw[:, 2:3], in_=tpsH[:, :],
            axis=mybir.AxisListType.X, op=mybir.AluOpType.max,
        )

        # ---- X extents from cnt_ps [16, 512] ----
        # NOTE: tensor_tensor_reduce and scalar_tensor_tensor (fused DVE
        # ISA ops) both crash the exec unit on this runtime path; use
        # plain compare/mul + reduce.
        colhit = small.tile([NPC, W], F32)
        nc.vector.tensor_scalar(
            colhit[:], cnt_ps[:], 0.5, None, mybir.AluOpType.is_ge
        )
        xprod = scratch.tile([NPC, W], F32, tag="xprod")
        nc.vector.tensor_mul(xprod[:], colhit[:], xlo_c[:])
        nc.vector.tensor_reduce(
            out=raw[:, 1:2], in_=xprod[:],
            axis=mybir.AxisListType.X, op=mybir.AluOpType.min,
        )
        xprod2 = scratch.tile([NPC, W], F32, tag="xprod")
        nc.vector.tensor_mul(xprod2[:], colhit[:], xhi_c[:])
        nc.vector.tensor_reduce(
            out=raw[:, 3:4], in_=xprod2[:],
            axis=mybir.AxisListType.X, op=mybir.AluOpType.max,
        )

        # lo_final = (lo_raw + 512) * (1 - nohit); hi_final = hi_raw + 512*nohit
        # where nohit = (hi_raw == 0). bbox layout: (ymin, xmin, ymax, xmax);
        # both lo (and both hi) columns are adjacent, so fix up 2-wide.
        bbox_f = small.tile([NPC, 4], F32)
        m2 = small.tile([NPC, 2], F32)
        nc.vector.tensor_scalar(m2[:], raw[:, 2:4], 0.0, None, mybir.AluOpType.is_equal)
        t2 = small.tile([NPC, 2], F32)
        nc.vector.tensor_scalar_add(t2[:], raw[:, 0:2], float(H))
        v2 = small.tile([NPC, 2], F32)
        nc.vector.tensor_mul(v2[:], t2[:], m2[:])
        nc.vector.tensor_sub(bbox_f[:, 0:2], t2[:], v2[:])
        w2 = small.tile([NPC, 2], F32)
        nc.vector.tensor_scalar_mul(w2[:], m2[:], float(H))
        nc.vector.tensor_add(bbox_f[:, 2:4], raw[:, 2:4], w2[:])

        bbox_i = small.tile([NPC, 4], I32)
        nc.vector.tensor_copy(bbox_i[:], bbox_f[:])
        nc.sync.dma_start(out=bbox_d, in_=bbox_i[:])

    nc.compile()
    return nc


def _consts():
    # one-hot for pair tile t: col t -> partitions 0-63 (image t),
    # col t+8 -> partitions 64-127 (image t+8)
    oneh = np.zeros((P, NPAIR * NPC), dtype=ml_dtypes.bfloat16)
    for t in range(NPAIR):
        oneh[0:64, t * NPC + t] = 1.0
        oneh[64:P, t * NPC + t + NPAIR] = 1.0
    ident = np.eye(P, dtype=np.float32)
    f = np.arange(W, dtype=np.float32)
    xlo = np.broadcast_to(f - W, (NPC, W)).copy()
    xhi = np.broadcast_to(f + 1, (NPC, W)).copy()
    # pair tile subrow j on partition p is image row r = 8*(p%64) + j
    p = np.arange(P)
    j = np.arange(JROWS)
    r = (8 * (p[:, None] % 64) + j[None, :]).astype(np.float32)  # [128, 8]
    # layout is (t j): col t*8 + j, same [128, 8] block for every t;
    # masked per partition half (s=0: images 0-7 on p<64, s=1: 8-15)
    lowmask = (p[:, None] < 64).astype(np.float32)
    ylo = np.tile(r - H, (1, NPAIR)).astype(np.float32)
    yhi = np.tile(r + 1, (1, NPAIR)).astype(np.float32)
    ylos = [ylo * lowmask, ylo * (1 - lowmask)]
    yhis = [yhi * lowmask, yhi * (1 - lowmask)]
    return oneh, ident, xlo, xhi, ylos, yhis


def kernel(mask):
    global _compiled, LAST_RESULTS
    mask = np.ascontiguousarray(np.asarray(mask), dtype=np.float32)
    assert mask.shape == (N, 1, H, W), mask.shape
    if _compiled is None:
        _compiled = _build_nc()
    nc = _compiled
    oneh, ident, xlo, xhi, ylos, yhis = _consts()
    m = mask.reshape(N, H, W)
    in_maps = []
    for c in range(N_CORES):
        in_maps.append({
            "mask": np.ascontiguousarray(
                m[c * NPC:(c + 1) * NPC].reshape(NPC * H, W)
            ),
            "onehot": oneh,
            "ident": ident,
            "xlo_const": xlo,
            "xhi_const": xhi,
            "ylo_const0": ylos[0],
            "ylo_const1": ylos[1],
            "yhi_const0": yhis[0],
            "yhi_const1": yhis[1],
        })
    res = run_bass_kernel_spmd(nc, in_maps, list(range(N_CORES)), trace=TRACE)
    LAST_RESULTS = res
    out = np.concatenate([res.results[c]["bbox"] for c in range(N_CORES)], axis=0)
    return out.astype(np.int32, copy=False)


# revision 18
# speedup vs baseline: 1.5007x; 1.5007x over previous
"""Bounding-box kernel for Trainium2 (Bass/Tile), 8-core SPMD.

Problem: mask [128, 1, 512, 512] f32 -> bbox [128, 4] int32
  (y_min, x_min, y_max, x_max) of the region where mask >= 0.5,
  with (0, 0, H, W) when a row/col has no hit.

Strategy (per core, 16 images, single qSync HWDGE queue):
  - DMA each image [512, 512] as one [128, 4, 512] tile (partition p
    holds rows 4p..4p+3 -> contiguous 8KB descriptors, the per-engine
    throughput sweet spot: ~26.5 GB/s x 16 engines ~ 424 GB/s).
  - Threshold: ACT computes h = Relu(x*2^25 - (2^24-1)) in bf16, which
    is exactly 0 iff x < 0.5 and >= 1 otherwise (exact for every f32).
  - Column extents: one-hot [128, 16] lhsT matmuls accumulate per-image
    column hit-mass into PSUM [16, 512] (partition = image); gpsimd
    pre-adds block pairs to halve the matmul count; then compare/mul/
    reduce on DVE.
  - Row extents stay in [128, *] space: rowmax over W of h (bf16)
    -> [128, 64] (col = i*4 + b, image row r = 4p + b), compare, mul
    by index consts, reduce over b -> [128, 16] (col = image), one PE
    transpose -> [16, 128] PSUM, full-partition reduce. This chain
    runs on gpsimd at the tail, in parallel with the X chain on DVE.
  - Last image: two [128, 2, 512] half loads (4KB descriptors) so the
    final arrival -> answer chain is short.
"""

import numpy as np
import ml_dtypes
from contextlib import ExitStack

import concourse.bass as bass
import concourse.bacc as bacc
import concourse.tile as tile
import concourse.mybir as mybir
from concourse.bass_utils import run_bass_kernel_spmd

N_CORES = 8
N, H, W = 128, 512, 512
NPC = N // N_CORES          # images per core = 16
P = 128                     # SBUF partitions
NBLK = H // P               # 4 row blocks per image
F32 = mybir.dt.float32
BF16 = mybir.dt.bfloat16
I32 = mybir.dt.int32

# Relu(x * 2^25 - (2^24 - 1)) == 0 iff x < 0.5, >= 1 iff x >= 0.5, exact
# for EVERY f32 x: x*2^25 is exact (power-of-2 scale); for x < 0.5,
# x*2^25 <= 2^24 - 1 so the true sum is <= 0 (rounding is monotone, 0 is
# representable); for x >= 0.5 the true sum is >= 1 and rounds to >= 1.
ACT_SCALE = float(2**25)
ACT_BIAS = float(1 - 2**24)

TRACE = False               # test.py sets True to capture a HW profile
LAST_RESULTS = None         # BassKernelResults of the last run

_compiled = None


def _build_nc():
    nc = bacc.Bacc(
        "TRN2", target_bir_lowering=False, debug=False, num_devices=N_CORES
    )
    mask_d = nc.dram_tensor("mask", [NPC * H, W], F32, kind="ExternalInput").ap()
    oneh_d = nc.dram_tensor("onehot", [P, NPC * NPC], BF16, kind="ExternalInput").ap()
    ident_d = nc.dram_tensor("ident", [P, P], F32, kind="ExternalInput").ap()
    xlo_d = nc.dram_tensor("xlo_const", [NPC, W], F32, kind="ExternalInput").ap()
    xhi_d = nc.dram_tensor("xhi_const", [NPC, W], F32, kind="ExternalInput").ap()
    ylo_d = nc.dram_tensor("ylo_const", [P, NPC * NBLK], F32, kind="ExternalInput").ap()
    yhi_d = nc.dram_tensor("yhi_const", [P, NPC * NBLK], F32, kind="ExternalInput").ap()
    bbox_d = nc.dram_tensor("bbox", [NPC, 4], I32, kind="ExternalOutput").ap()

    with tile.TileContext(nc) as tc, ExitStack() as ctx:
        consts = ctx.enter_context(tc.tile_pool(name="consts", bufs=1))
        xpool = ctx.enter_context(tc.tile_pool(name="x", bufs=8))
        hpool = ctx.enter_context(tc.tile_pool(name="h", bufs=16))
        hspool = ctx.enter_context(tc.tile_pool(name="hs", bufs=8))
        lastpool = ctx.enter_context(tc.tile_pool(name="last", bufs=2))
        small = ctx.enter_context(tc.tile_pool(name="small", bufs=1))
        scratch = ctx.enter_context(tc.tile_pool(name="scratch", bufs=2))
        psum = ctx.enter_context(tc.tile_pool(name="psum", bufs=1, space="PSUM"))

        # pin const loads to the start of the schedule; the tile
        # scheduler otherwise sinks tail-only consts (ylo/yhi/xlo/xhi)
        # next to their consumers, adding their DMA latency to the tail
        with tc.high_priority():
            oneh = consts.tile([P, NPC * NPC], BF16)
            nc.scalar.dma_start(out=oneh[:], in_=oneh_d)
            ident = consts.tile([P, P], F32)
            nc.scalar.dma_start(out=ident[:], in_=ident_d)
            xlo_c = consts.tile([NPC, W], F32)
            nc.scalar.dma_start(out=xlo_c[:], in_=xlo_d)
            xhi_c = consts.tile([NPC, W], F32)
            nc.scalar.dma_start(out=xhi_c[:], in_=xhi_d)
            ylo_c = consts.tile([P, NPC * NBLK], F32)
            nc.scalar.dma_start(out=ylo_c[:], in_=ylo_d)
            yhi_c = consts.tile([P, NPC * NBLK], F32)
            nc.scalar.dma_start(out=yhi_c[:], in_=yhi_d)
            act_bias = consts.tile([P, 1], F32)
            nc.vector.memset(act_bias[:], ACT_BIAS)

        # rowmax[p, i*4 + b] = max over w of image i's block b on
        # partition p (image row r = 4p + b), in bf16
        rowmax = small.tile([P, NPC * NBLK], BF16)
        rowmax_v = rowmax.rearrange("p (i b) -> p i b", i=NPC)
        cnt_ps = psum.tile([NPC, W], F32)    # per-image column hit-mass
        tpsL = psum.tile([NPC, P], F32)      # transposed y-lo stage
        tpsH = psum.tile([NPC, P], F32)      # transposed y-hi stage

        for i in range(NPC - 1):
            x = xpool.tile([P, NBLK, W], F32, tag="x")
            nc.sync.dma_start(
                out=x[:],
                in_=mask_d[i * H:(i + 1) * H, :].rearrange("(p b) w -> p b w", p=P),
            )
            h = hpool.tile([P, NBLK, W], BF16, tag="h")
            nc.scalar.activation(
                h[:], x[:], mybir.ActivationFunctionType.Relu,
                bias=act_bias[:], scale=ACT_SCALE,
            )
            nc.vector.tensor_reduce(
                out=rowmax_v[:, i, :], in_=h[:],
                axis=mybir.AxisListType.X, op=mybir.AluOpType.max,
            )
            lhsT = oneh[:, i * NPC:(i + 1) * NPC]
            if i < NPC - 2:
                # pre-sum block pairs on gpsimd: halves the PE matmul
                # count (hit-mass stays 0 iff no hit)
                h_v = h.rearrange("p (m q) w -> p m q w", q=2)
                hs = hspool.tile([P, 2, W], BF16)
                nc.gpsimd.tensor_add(hs[:], h_v[:, :, 0, :], h_v[:, :, 1, :])
                for m in range(2):
                    nc.tensor.matmul(
                        cnt_ps[:, :], lhsT, hs[:, m, :],
                        start=(i == 0 and m == 0), stop=False,
                    )
            else:
                # penultimate image: skip the gpsimd hop (shorter tail)
                for b in range(NBLK):
                    nc.tensor.matmul(
                        cnt_ps[:, :], lhsT, h[:, b, :],
                        start=False, stop=False,
                    )

        # last image: two half loads so its compute chain starts while
        # the second half is still in flight
        i = NPC - 1
        lhsT = oneh[:, i * NPC:(i + 1) * NPC]
        for u in range(2):
            x = lastpool.tile([P, 2, W], F32, tag="xh")
            nc.sync.dma_start(
                out=x[:],
                in_=mask_d[i * H:(i + 1) * H, :]
                .rearrange("(p b) w -> p b w", p=P)[:, 2 * u:2 * u + 2, :],
            )
            h = lastpool.tile([P, 2, W], BF16, tag="hh")
            nc.scalar.activation(
                h[:], x[:], mybir.ActivationFunctionType.Relu,
                bias=act_bias[:], scale=ACT_SCALE,
            )
            nc.vector.tensor_reduce(
                out=rowmax_v[:, i, 2 * u:2 * u + 2], in_=h[:],
                axis=mybir.AxisListType.X, op=mybir.AluOpType.max,
            )
            for b in range(2):
                nc.tensor.matmul(
                    cnt_ps[:, :], lhsT, h[:, b, :],
                    start=False, stop=(u == 1 and b == 1),
                )

        # ---- Y extents in [128, *] space, on gpsimd (DVE runs X) ----
        rowhit = small.tile([P, NPC * NBLK], F32)
        nc.gpsimd.tensor_scalar(
            rowhit[:], rowmax[:], 0.5, None, mybir.AluOpType.is_ge
        )
        # loI[:, i] = min over b of rowhit*(r-512); 0 if no hit (and 0
        # is neutral for the final min since hits give negatives)
        loI = small.tile([P, NPC], F32)
        hiI = small.tile([P, NPC], F32)
        prod = scratch.tile([P, NPC * NBLK], F32, tag="yprod")
        nc.gpsimd.tensor_mul(prod[:], rowhit[:], ylo_c[:])
        prod_v = prod.rearrange("p (i b) -> p i b", i=NPC)
        nc.vector.tensor_reduce(
            out=loI[:], in_=prod_v[:],
            axis=mybir.AxisListType.X, op=mybir.AluOpType.min,
        )
        prod2 = scratch.tile([P, NPC * NBLK], F32, tag="yprod")
        nc.gpsimd.tensor_mul(prod2[:], rowhit[:], yhi_c[:])
        prod2_v = prod2.rearrange("p (i b) -> p i b", i=NPC)
        nc.vector.tensor_reduce(
            out=hiI[:], in_=prod2_v[:],
            axis=mybir.AxisListType.X, op=mybir.AluOpType.max,
        )
        # transpose [128, 16] -> [16, 128], partition = image
        nc.tensor.matmul(
            tpsL[:, :], loI[:], ident[:],
            is_transpose=True, start=True, stop=True,
        )
        nc.tensor.matmul(
            tpsH[:, :], hiI[:], ident[:],
            is_transpose=True, start=True, stop=True,
        )

        # raw extents tile: col 0 = ylo, 1 = xlo, 2 = yhi, 3 = xhi
        # (lo values are lo-512 for hit, 0 for none; hi are hi+1 or 0)
        raw = small.tile([NPC, 4], F32)
        nc.vector.tensor_reduce(
            out=raw[:, 0:1], in_=tpsL[:, :],
            axis=mybir.AxisListType.X, op=mybir.AluOpType.min,
        )
        nc.vector.tensor_reduce(
            out=raw[:, 2:3], in_=tpsH[:, :],
            axis=mybir.AxisListType.X, op=mybir.AluOpType.max,
        )

        # ---- X extents from cnt_ps [16, 512] on DVE ----
        # NOTE: tensor_tensor_reduce and scalar_tensor_tensor (fused DVE
        # ISA ops) both crash the exec unit on this runtime path; use
        # plain compare/mul + reduce.
        colhit = small.tile([NPC, W], F32)
        nc.vector.tensor_scalar(
            colhit[:], cnt_ps[:], 0.5, None, mybir.AluOpType.is_ge
        )
        xprod = scratch.tile([NPC, W], F32, tag="xprod")
        nc.vector.tensor_mul(xprod[:], colhit[:], xlo_c[:])
        nc.vector.tensor_reduce(
            out=raw[:, 1:2], in_=xprod[:],
            axis=mybir.AxisListType.X, op=mybir.AluOpType.min,
        )
        xprod2 = scratch.tile([NPC, W], F32, tag="xprod")
        nc.vector.tensor_mul(xprod2[:], colhit[:], xhi_c[:])
        nc.vector.tensor_reduce(
            out=raw[:, 3:4], in_=xprod2[:],
            axis=mybir.AxisListType.X, op=mybir.AluOpType.max,
        )

        # fixup: gm = (hi_raw > 0) * 512 (hit indicator scaled);
        # lo_final = lo_raw + gm   (hit: ymin-512+512 = ymin; none: 0)
        # hi_final = hi_raw + 512 - gm  (hit: hi_raw; none: 512)
        bbox_f = small.tile([NPC, 4], F32)
        gm = small.tile([NPC, 2], F32)
        nc.vector.tensor_scalar(
            gm[:], raw[:, 2:4], 0.0, float(H),
            mybir.AluOpType.is_gt, mybir.AluOpType.mult,
        )
        nc.vector.tensor_add(bbox_f[:, 0:2], raw[:, 0:2], gm[:])
        t5 = small.tile([NPC, 2], F32)
        nc.vector.tensor_scalar_add(t5[:], raw[:, 2:4], float(H))
        nc.vector.tensor_sub(bbox_f[:, 2:4], t5[:], gm[:])

        bbox_i = small.tile([NPC, 4], I32)
        nc.vector.tensor_copy(bbox_i[:], bbox_f[:])
        nc.sync.dma_start(out=bbox_d, in_=bbox_i[:])

    nc.compile()
    return nc


def _consts():
    oneh = np.zeros((P, NPC * NPC), dtype=ml_dtypes.bfloat16)
    for i in range(NPC):
        oneh[:, i * NPC + i] = 1.0
    ident = np.eye(P, dtype=np.float32)
    f = np.arange(W, dtype=np.float32)
    xlo = np.broadcast_to(f - W, (NPC, W)).copy()
    xhi = np.broadcast_to(f + 1, (NPC, W)).copy()
    # block b on partition p is image row r = 4p + b, layout (i b)
    p = np.arange(P)
    b = np.arange(NBLK)
    r = (NBLK * p[:, None] + b[None, :]).astype(np.float32)  # [128, 4]
    ylo = np.tile(r - H, (1, NPC)).astype(np.float32)
    yhi = np.tile(r + 1, (1, NPC)).astype(np.float32)
    return oneh, ident, xlo, xhi, ylo, yhi


def kernel(mask):
    global _compiled, LAST_RESULTS
    mask = np.ascontiguousarray(np.asarray(mask), dtype=np.float32)
    assert mask.shape == (N, 1, H, W), mask.shape
    if _compiled is None:
        _compiled = _build_nc()
    nc = _compiled
    oneh, ident, xlo, xhi, ylo, yhi = _consts()
    m = mask.reshape(N, H, W)
    in_maps = []
    for c in range(N_CORES):
        in_maps.append({
            "mask": np.ascontiguousarray(
                m[c * NPC:(c + 1) * NPC].reshape(NPC * H, W)
            ),
            "onehot": oneh,
            "ident": ident,
            "xlo_const": xlo,
            "xhi_const": xhi,
            "ylo_const": ylo,
            "yhi_const": yhi,
        })
    res = run_bass_kernel_spmd(nc, in_maps, list(range(N_CORES)), trace=TRACE)
    LAST_RESULTS = res
    out = np.concatenate([res.results[c]["bbox"] for c in range(N_CORES)], axis=0)
    return out.astype(np.int32, copy=False)
